# revision 23
# baseline (speedup 1.0000x reference)
"""Trainium2 Bass kernel for CIN: out[b,m,d] = sigmoid(einsum('bid,bjd,ijm', x0, x, K)).

Shapes (hardcoded): x0,x [4096, 40, 64] f32, kernel [40, 40, 128] f32,
out [4096, 128, 64] f32.

Sharding: data-parallel over batch B across 8 NeuronCores (512 b each).

Per-core pipeline (groups of 8 b's; free dim = 8*64 = 512), bf16 on the
engines, fp32 accumulation in PSUM.  The interaction tensor
Z[(i j), (b d)] = x0[i,(b d)] * x[j,(b d)] is built with (i j) on
partitions, blocked 3 i-rows per 128-partition chunk (14 chunks):

  - zin[p, c, bd] = x0T[3c + p//40, bd] (the replicated-x0 "A side"):
      * chunks 0..7  are HOST-replicated and DMA'd straight into zin
        (pure layout transform; DMA has headroom, PE/ACT do not)
      * chunks 8..13 come from 6 replication matmuls with constant 0/1
        weights (PSUM), evacuated to zin by the Scalar engine in 3
        pair-copies (PSUM -> SBUF bf16)
  - TWO DVE multiplies per group build zc = zin * bb with
    in1 = bb[128, 512] broadcast over the chunk axis (all operands SBUF
    bf16, unit inner stride -> DVE 2x mode): the host-chunk half (0..7)
    has no evac dependency so it starts the moment the previous multiply
    drains; only the shorter PE-chunk half (8..13) waits on the Scalar
    evacs.  This recovers most of the ~0.4us evac-gated handoff that a
    single fused multiply paid per group.
  - 14 accumulated matmuls  pso += K_c^T @ zc_c  (contraction (i j))
  - sigmoid fused into PSUM evacuation on ACT -> bf16, DMA out as
    [M, b, d]; host transposes back to [b, M, d] and widens to f32.

Issue order is software-pipelined with lookahead 3: loads prefetch 4
groups ahead (absorbs HBM jitter; aggregate in-stream is ~320 GB/s per
core), replication matmuls + evacs run 3 groups ahead, and the sigmoid
of group g is issued to the Scalar queue before the evacs of group g+3
so the pso pool never blocks the next mains.  Steady state is DVE-bound
at ~4.3us per group (giant TT 3.89us + handoff); PE ~3.4us, ACT ~4.0us,
with the packed replication pairs keeping the PE warm (HAM at 2.4 GHz).

Host-side prep (not on the HW critical path): inputs cast to bf16 and
packed so every DMA is a dense, partition-contiguous load.
"""

import sys

for _p in ("/opt/trn_rl_repo", "/root/.axon_site/_ro/trn_rl_repo"):
    if _p not in sys.path:
        sys.path.insert(0, _p)

from contextlib import ExitStack

import numpy as np
import ml_dtypes

import concourse.bass as bass
from concourse import bacc
import concourse.tile as tile
from concourse import mybir
from concourse.bass_utils import run_bass_kernel_spmd

B, F0, F, D, M = 4096, 40, 40, 64, 128
NCORES = 8
NB = B // NCORES            # 512 b per core
GB = 8                      # b's per group
FREE = GB * D               # 512 = matmul free dim = one PSUM bank (f32)
NG = NB // GB               # 64 groups per core
IPC = 3                     # i-rows per chunk
ROWS = IPC * F              # 120 valid rows per chunk
NCHUNK = (F0 + IPC - 1) // IPC  # 14
HOSTC = 8                   # chunks 0..HOSTC-1 replicated on the host
PEC = NCHUNK - HOSTC        # 6 chunks replicated on the PE (3 pairs)
PACK_REPS = True            # tile_position row-packing of rep pairs
LOOKAHEAD = 3

f32 = mybir.dt.float32
bf16 = mybir.dt.bfloat16
BF16 = ml_dtypes.bfloat16


def _pack_kernel(kernel_np: np.ndarray) -> np.ndarray:
    """K[i,j,m] -> kwT [128, NCHUNK, M] bf16,
    kwT[p, c, m] = K[3c + p//40, p%40, m] (zero where invalid)."""
    kf = np.zeros((NCHUNK, 128, M), dtype=np.float32)
    p = np.arange(ROWS)
    for c in range(NCHUNK):
        i = IPC * c + p // F
        valid = i < F0
        kf[c, p[valid]] = kernel_np[i[valid], p[valid] % F]
    return np.ascontiguousarray(kf.transpose(1, 0, 2).astype(BF16))


def _pack_reps() -> np.ndarray:
    """Constant replication weights [104, PEC//2, 2, 128] bf16 for the PE
    chunks (HOSTC..NCHUNK-1).  Slot [0:40, q, s] holds the weights for
    chunk HOSTC+2q+s (base-0 operands, unpacked mode); slot
    [64:104, q, 1] duplicates the odd chunk's weights so a packed pair
    can run as row-tiles (0,0) and (64,0)."""
    rp = np.zeros((104, PEC // 2, 2, 128), dtype=np.float32)
    p = np.arange(ROWS)
    for q in range(PEC // 2):
        for s in (0, 1):
            c = HOSTC + 2 * q + s
            i = IPC * c + p // F
            valid = i < F0
            rp[i[valid], q, s, p[valid]] = 1.0
            if s == 1:
                rp[64 + i[valid], q, s, p[valid]] = 1.0
    return np.ascontiguousarray(rp.astype(BF16))


def _pack_x(x0: np.ndarray) -> np.ndarray:
    """-> xp [NCORES, NG, 2, F0, FREE] bf16: x0T per (core, group),
    duplicated so a copy can sit at partitions 64:104 for packed reps."""
    x0r = x0.reshape(NCORES, NG, GB, F0, D).transpose(0, 1, 3, 2, 4)
    x0r = x0r.reshape(NCORES, NG, F0, FREE).astype(BF16)
    return np.ascontiguousarray(
        np.broadcast_to(x0r[:, :, None], (NCORES, NG, 2, F0, FREE)))


def _pack_b(x: np.ndarray) -> np.ndarray:
    """-> bp [NCORES, NG, 128, FREE] bf16: B[p, bd] = xT[p%40, bd] for
    p < 120, zero pad rows."""
    xr = x.reshape(NCORES, NG, GB, F, D).transpose(0, 1, 3, 2, 4)
    xr = xr.reshape(NCORES, NG, F, FREE).astype(BF16)
    bp = np.zeros((NCORES, NG, 128, FREE), dtype=BF16)
    bp[:, :, 0:ROWS, :] = np.concatenate([xr] * IPC, axis=2)
    return bp


def _pack_hostrep(x0: np.ndarray) -> np.ndarray:
    """-> hp [NCORES, NG, 128, HOSTC, FREE] bf16:
    hp[.., p, c, bd] = x0T[3c + p//40, bd] for p < 120, zero pad rows."""
    x0r = x0.reshape(NCORES, NG, GB, F0, D).transpose(0, 1, 3, 2, 4)
    x0r = np.ascontiguousarray(x0r.reshape(NCORES, NG, F0, FREE)).astype(BF16)
    hp = np.zeros((NCORES, NG, 128, HOSTC, FREE), dtype=BF16)
    p = np.arange(ROWS)
    for c in range(HOSTC):
        hp[:, :, 0:ROWS, c, :] = x0r[:, :, IPC * c + p // F, :]
    return hp


def _build(nb: int):
    ng = nb // GB

    nc = bacc.Bacc("TRN2", num_devices=8)
    xp = nc.declare_dram_parameter("xp", [ng, 2, F0, FREE], bf16, isOutput=False)
    bpp = nc.declare_dram_parameter("bp", [ng, 128, FREE], bf16, isOutput=False)
    hpp = nc.declare_dram_parameter("hp", [ng, 128, HOSTC, FREE], bf16,
                                    isOutput=False)
    kp = nc.declare_dram_parameter("kp", [128, NCHUNK, M], bf16, isOutput=False)
    rep = nc.declare_dram_parameter("rep", [104, PEC // 2, 2, 128], bf16,
                                    isOutput=False)
    outp = nc.declare_dram_parameter("out", [M, nb, D], bf16, isOutput=True)

    with ExitStack() as ctx:
        tc = ctx.enter_context(tile.TileContext(nc))
        singles = ctx.enter_context(tc.tile_pool(name="singles", bufs=1))
        xx_pool = ctx.enter_context(tc.tile_pool(name="xx", bufs=6))
        bb_pool = ctx.enter_context(tc.tile_pool(name="bb", bufs=6))
        zin_pool = ctx.enter_context(tc.tile_pool(name="zin", bufs=6))
        zc_pool = ctx.enter_context(tc.tile_pool(name="zc", bufs=3))
        osb_pool = ctx.enter_context(tc.tile_pool(name="osb", bufs=3))
        psa_pool = ctx.enter_context(tc.tile_pool(name="psa", bufs=3, space="PSUM"))
        pso_pool = ctx.enter_context(tc.tile_pool(name="pso", bufs=2, space="PSUM"))

        kw = singles.tile([128, NCHUNK, M], bf16)
        nc.sync.dma_start(out=kw, in_=kp[:])
        rp = singles.tile([104, PEC // 2, 2, 128], bf16)
        nc.sync.dma_start(out=rp, in_=rep[:])

        # HAM warm-up spin: dense back-to-back matmuls raise the PE
        # clock-gate toward 2.4 GHz while the first groups' DMAs land.
        spin_w = singles.tile([128, 128], bf16)
        nc.vector.memset(spin_w, 0.0)
        spin_r = singles.tile([128, FREE], bf16)
        nc.vector.memset(spin_r, 0.0)
        ps_spin = pso_pool.tile([128, FREE], f32, tag="pso")
        for _ in range(12):
            nc.tensor.matmul(ps_spin, spin_w, spin_r, start=True, stop=True)

        xxs = [None] * ng
        bbs = [None] * ng
        zins = [None] * ng

        def load(g):
            xx = xx_pool.tile([128, FREE], bf16, tag="xx")
            nc.sync.dma_start(out=xx[0:F0, :], in_=xp[g, 0])
            if PACK_REPS:
                nc.sync.dma_start(out=xx[64:64 + F0, :], in_=xp[g, 1])
            bb = bb_pool.tile([128, FREE], bf16, tag="bb")
            nc.sync.dma_start(out=bb, in_=bpp[g])
            zin = zin_pool.tile([128, NCHUNK, FREE], bf16, tag="zin")
            nc.sync.dma_start(out=zin[:, 0:HOSTC, :], in_=hpp[g])
            xxs[g], bbs[g], zins[g] = xx, bb, zin

        def reps(g):
            xx, zin = xxs[g], zins[g]
            for q in range(PEC // 2):
                psa = psa_pool.tile([128, 2, FREE], f32, tag="psa")
                nc.tensor.matmul(psa[:, 0, :], rp[0:F0, q, 0, :],
                                 xx[0:F0, :], start=True, stop=True)
                if PACK_REPS:
                    nc.tensor.matmul(psa[:, 1, :], rp[64:64 + F0, q, 1, :],
                                     xx[64:64 + F0, :], start=True, stop=True)
                else:
                    nc.tensor.matmul(psa[:, 1, :], rp[0:F0, q, 1, :],
                                     xx[0:F0, :], start=True, stop=True)
                c = HOSTC + 2 * q
                nc.scalar.copy(out=zin[:, c:c + 2, :], in_=psa)

        def mult(g):
            # split multiply: the host-chunk half has NO evac dependency,
            # so the DVE starts it the moment the previous TT drains; only
            # the shorter PE-chunk half waits on the Scalar evacs.
            zc = zc_pool.tile([128, NCHUNK, FREE], bf16, tag="zc")
            nc.vector.tensor_tensor(
                out=zc[:, 0:HOSTC, :], in0=zins[g][:, 0:HOSTC, :],
                in1=bbs[g].unsqueeze(1).broadcast_to((128, HOSTC, FREE)),
                op=mybir.AluOpType.mult)
            nc.vector.tensor_tensor(
                out=zc[:, HOSTC:NCHUNK, :], in0=zins[g][:, HOSTC:NCHUNK, :],
                in1=bbs[g].unsqueeze(1).broadcast_to(
                    (128, NCHUNK - HOSTC, FREE)),
                op=mybir.AluOpType.mult)
            return zc

        zcs = [None] * ng

        def mains(g):
            zc = zcs[g]
            pso = pso_pool.tile([128, FREE], f32, tag="pso")
            for c in range(NCHUNK):
                nc.tensor.matmul(pso, kw[:, c, :], zc[:, c, :],
                                 start=(c == 0), stop=(c == NCHUNK - 1))
            osb = osb_pool.tile([128, GB, D], bf16, tag="osb")
            nc.scalar.activation(osb.rearrange("m b d -> m (b d)"), pso,
                                 mybir.ActivationFunctionType.Sigmoid)
            # out-DMA on the (otherwise idle) gpsimd queue: keeps the sync
            # queue's prefetch loads free of head-of-line blocking behind
            # sigmoid-dependent stores.
            nc.gpsimd.dma_start(out=outp[:, g * GB:(g + 1) * GB, :], in_=osb)

        for g in range(min(LOOKAHEAD + 1, ng)):
            load(g)
        for g in range(min(LOOKAHEAD, ng)):
            reps(g)
        for g in range(ng):
            if g + LOOKAHEAD + 1 < ng:
                load(g + LOOKAHEAD + 1)
            zcs[g] = mult(g)
            mains(g)
            # reps (and their ACT evacs) AFTER mains(g): the sigmoid(g)
            # lands on the ACT queue ahead of the evacs(g+2), so mains(g+2)
            # is never blocked behind a late sigmoid via the pso pool.
            if g + LOOKAHEAD < ng:
                reps(g + LOOKAHEAD)

    nc.finalize()
    return nc


_NC_CACHE = {}


def _get_nc():
    if "nc" not in _NC_CACHE:
        _NC_CACHE["nc"] = _build(NB)
    return _NC_CACHE["nc"]


def _make_in_maps(x0: np.ndarray, x: np.ndarray, kernel: np.ndarray):
    x0 = np.ascontiguousarray(np.asarray(x0, dtype=np.float32))
    x = np.ascontiguousarray(np.asarray(x, dtype=np.float32))
    kw = _pack_kernel(np.asarray(kernel, dtype=np.float32))
    rp = _pack_reps()
    xp = _pack_x(x0)
    bp = _pack_b(x)
    hp = _pack_hostrep(x0)
    return [
        {"xp": xp[i], "bp": bp[i], "hp": hp[i], "kp": kw, "rep": rp}
        for i in range(NCORES)
    ]


def kernel(x0: np.ndarray, x: np.ndarray, kernel: np.ndarray) -> np.ndarray:
    nc = _get_nc()
    in_maps = _make_in_maps(x0, x, kernel)
    res = run_bass_kernel_spmd(nc, in_maps, list(range(NCORES)))
    outs = [
        np.asarray(r["out"]).astype(np.float32).transpose(1, 0, 2)
        for r in res.results
    ]
    return np.ascontiguousarray(np.concatenate(outs, axis=0))
